# revision 55
# baseline (speedup 1.0000x reference)
"""Trainium2 Bass kernel for nn_Expansion (e3nn-style tensor-product expansion).

Math reformulation (verified against the jax reference to ~5e-3):
  h   = silu(node_emb @ lw1 + lb1)                         [B,64]
  hb  = silu(node_emb @ bw1 + bb1)                         [B,64]
  x0  = feat[:,:128] @ W0 / sqrt(128)                      [B,16]
  x1k = feat[:,128+k::3] @ W1 / 8          (k=0,1,2)       [B,16]

The per-sample path contractions are a batched bilinear form over the outer
product z[b,(c,w)] = h[b,c]*x[b,w] (K = 64*16 = 1024) against reshaped weight
matrices R built from lw2 on the host; the lb2 row and the bias MLP are folded
in as extra K=16 / K=65 accumulation chunks.  This avoids materializing
w = h@lw2 ([B,36864], ~600 MB) entirely.

Sharding: pure data parallel, batch 4096 -> 8 cores x 512.  Weights replicated.

Device layout per core (B_c = 512, all matmul operands bf16):
  - h is produced DIRECTLY in partition-replicated form: the prep matmuls use
    host-replicated weight matrices (lw1_rep / W0_rep / W1_rep), so each
    [128, B_c] pre-activation PSUM tile is already laid out as the z K-chunk
    (p = 16*c8 + w).  One silu/copy per tile moves it to SBUF as bf16; no
    separate replication pass.
  - z chunks are DVE multiplies of the replicated tiles (bf16, 2x mode).
  - Mains are emitted BANK-MAJOR (for each of 9 output banks, all 4 b-tiles
    in 2 j-pairs), chunk-major inside a pair, so the R-weight DMA stream is
    consumed at ~1/4 the peak rate (no stalls waiting on weight loads) and
    the z chunks are consumed right as the prep pipeline produces them.
  - Output is written COMPACT ([B_c, 4352] f32: blk00 | (blk01_k blk10_k)*3 |
    blk11-diag) via one contiguous PSUM->SBUF copy + one DMA per psum pair;
    the 80x80 block structure (1o interleave, blk11 diagonal replication,
    zero fill) is reassembled on the host after the gather.
All path normalization constants are folded into the host-side weight prep.
"""

import sys

import numpy as np

sys.path.insert(0, "/opt/trn_rl_repo")

import ml_dtypes  # noqa: E402

B_TOTAL = 4096
N_CORES = 8
BC = B_TOTAL // N_CORES  # 512 samples per core
P = 128
NB = BC // P  # 4 b-tiles per core
C3 = 1.0 / np.sqrt(3.0)
NCOL = 4352  # compact output columns per sample

MM_MODE = "bf16"
BF16 = ml_dtypes.bfloat16

_CACHE = {}


def _build_program(skip_lb2, skip_b1):
    import concourse.tile as tile
    from concourse import bacc, mybir

    F32 = mybir.dt.float32
    MM = mybir.dt.bfloat16

    nc = bacc.Bacc("TRN2", target_bir_lowering=False, debug=False,
                   num_devices=N_CORES)

    t = {}
    # consolidated small inputs (fewer DMAs -> less serial HWDGE at startup):
    # WS0a [128,640] = lw1r-q0 (128) | embT (512)   (hbc0 critical path)
    # WS0b [128,640] = W0r (128) | feats (512)      (xbc0 critical path)
    # WS1a [128,128] = lw1r-q1 (tiny: unblocks the hbc1 matmul early)
    # WS1b [128,832] = lw1r-q2..7 (768) | bw1 (64)
    # WS2 [64,1664]  = W1r (128) | featv0 | featv1 | featv2 (512 each)
    # WF  [128,9] f32 = lb1r (8) | bb1 (col 8, rows 0:64)
    t["WS0a"] = nc.dram_tensor("WS0a", [P, 640], MM, kind="ExternalInput").ap()
    t["WS0b"] = nc.dram_tensor("WS0b", [P, 640], MM, kind="ExternalInput").ap()
    t["WS1a"] = nc.dram_tensor("WS1a", [P, 128], MM, kind="ExternalInput").ap()
    t["WS1b"] = nc.dram_tensor("WS1b", [P, 832], MM, kind="ExternalInput").ap()
    t["WS2"] = nc.dram_tensor("WS2", [64, 1664], MM, kind="ExternalInput").ap()
    t["WF"] = nc.dram_tensor("WF", [P, 9], F32, kind="ExternalInput").ap()
    t["R0"] = nc.dram_tensor("R0", [1024, 1280], MM, kind="ExternalInput").ap()
    t["R1"] = nc.dram_tensor("R1", [1024, 1024], MM, kind="ExternalInput").ap()
    t["BB"] = nc.dram_tensor("BB", [65, 1280], MM, kind="ExternalInput").ap()
    if not skip_lb2:
        t["R0LB"] = nc.dram_tensor("R0LB", [16, 1280], MM, kind="ExternalInput").ap()
        t["R1LB"] = nc.dram_tensor("R1LB", [16, 1024], MM, kind="ExternalInput").ap()
    t["out"] = nc.dram_tensor("out", [BC, NCOL], F32, kind="ExternalOutput").ap()

    with tile.TileContext(nc) as tc:
        _emit(tc, t, skip_lb2, skip_b1, mybir, MM, F32)

    nc.compile()
    return nc


def _emit(tc, t, skip_lb2, skip_b1, mybir, MM, F32):
    nc = tc.nc
    AF = mybir.ActivationFunctionType
    from contextlib import ExitStack

    bsl = [slice(P * j, P * (j + 1)) for j in range(NB)]

    with ExitStack() as ctx:
        wpool = ctx.enter_context(tc.tile_pool(name="weights", bufs=1))
        apool = ctx.enter_context(tc.tile_pool(name="acts", bufs=1))
        zpool = ctx.enter_context(tc.tile_pool(name="z", bufs=1))
        spool = ctx.enter_context(tc.tile_pool(name="stage", bufs=5))
        # PSUM: prep ring 2x[128,512] (2 banks) + mains ring 3x[128,1024]
        # (6 banks) = 8 banks exactly
        prep_psum = ctx.enter_context(tc.tile_pool(name="prep_psum", bufs=2, space="PSUM"))
        main_psum = ctx.enter_context(tc.tile_pool(name="main_psum", bufs=3, space="PSUM"))

        # ---- SBUF tiles ----
        ws0a_sb = wpool.tile([P, 640], MM, tag="ws0a")
        ws0b_sb = wpool.tile([P, 640], MM, tag="ws0b")
        ws1a_sb = wpool.tile([P, 128], MM, tag="ws1a")
        ws1b_sb = wpool.tile([P, 832], MM, tag="ws1b")
        ws2_sb = wpool.tile([64, 1664], MM, tag="ws2")
        wf_sb = wpool.tile([P, 9], F32, tag="wf")
        act_warm = wpool.tile([1, 4], F32, tag="act_warm")
        # views into the packed tiles
        lw1r_q = [ws0a_sb[:, 0:128], ws1a_sb[:, 0:128]] + [
            ws1b_sb[:, 128 * i:128 * (i + 1)] for i in range(6)]
        emb_sb = ws0a_sb[:, 128:640]
        W0r_sb = ws0b_sb[:, 0:128]
        feats_sb = ws0b_sb[:, 128:640]
        bw1_sb = ws1b_sb[:, 768:832]
        W1r_sb = ws2_sb[:, 0:128]
        featv_sb = [ws2_sb[:, 128 + 512 * k:640 + 512 * k] for k in range(3)]
        lb1r_sb = wf_sb[:, 0:8]
        bb1_sb = wf_sb[0:64, 8:9]

        def bias_kw(q):
            return {} if skip_b1 else {"bias": lb1r_sb[:, q:q + 1]}
        R0_sb = wpool.tile([P, 8, 1280], MM, tag="R0")
        R1_sb = wpool.tile([P, 8, 1024], MM, tag="R1")
        BB_sb = wpool.tile([65, 1280], MM, tag="BB")
        if not skip_lb2:
            R0LB_sb = wpool.tile([16, 1280], MM, tag="R0LB")
            R1LB_sb = wpool.tile([16, 1024], MM, tag="R1LB")
        hbc = [apool.tile([P, BC], MM, name=f"hbc{q}", tag=f"hbc{q}")
               for q in range(8)]
        xbc = [apool.tile([P, BC], MM, name=f"xbc{k}", tag=f"xbc{k}")
               for k in range(4)]
        hbp_sb = apool.tile([65, BC], MM, tag="hbp")
        z = [[zpool.tile([P, BC], MM, name=f"z{tdx}_{q}", tag=f"z{tdx}_{q}")
              for q in range(8)] for tdx in range(4)]

        # constant hbp row + Silu table preload FIRST (no input deps; these
        # must precede the Pool SWDGE DMAs in the Pool queue, and the 1.3us
        # LoadActFuncSet must be off the hbc0 critical path)
        nc.gpsimd.memset(hbp_sb[64:65, :], 1.0)
        nc.gpsimd.memset(act_warm[:], 0.0)
        nc.scalar.activation(act_warm[:, 2:4], act_warm[:, 0:2], AF.Silu)

        # ---- input DMAs, ordered by first use on the critical path ----
        # WS0 feeds the hbc0 chain; WS1/WS2 feed xbc0 and the rest of prep;
        # R0 cols 0:512 / 1024:1280 stream per-q so the first bank (p00a
        # interleaved with p11) can start on chunk 0 and stay fed.
        r0v = t["R0"].rearrange("(q p) n -> p q n", p=P)
        r1v = t["R1"].rearrange("(q p) n -> p q n", p=P)
        nc.sync.dma_start(ws0a_sb[:], t["WS0a"][:])
        nc.sync.dma_start(ws0b_sb[:], t["WS0b"][:])
        nc.sync.dma_start(ws1a_sb[:], t["WS1a"][:])
        nc.sync.dma_start(ws1b_sb[:], t["WS1b"][:])
        # WF (silu bias) and the first two R0 chunks go through the idle Pool
        # engine's SWDGE queue: their descriptor generation runs in parallel
        # with the serial HWDGE generation chain of the WS* DMAs above
        if not skip_b1:
            nc.gpsimd.dma_start(wf_sb[:], t["WF"][:])
        for q in (0, 1):
            nc.gpsimd.dma_start(R0_sb[:, q, 0:512], r0v[:, q, 0:512])
        for q0, q1 in ((2, 4), (4, 6), (6, 8)):
            nc.sync.dma_start(R0_sb[:, q0:q1, 0:512], r0v[:, q0:q1, 0:512])
        nc.sync.dma_start(ws2_sb[:], t["WS2"][:])
        nc.sync.dma_start(BB_sb[:], t["BB"][:])
        nc.sync.dma_start(R0_sb[:, :, 512:1024], r0v[:, :, 512:1024])
        nc.sync.dma_start(R1_sb[:, :, 0:512], r1v[:, :, 0:512])
        nc.sync.dma_start(R1_sb[:, :, 512:1024], r1v[:, :, 512:1024])
        nc.sync.dma_start(R0_sb[:, :, 1024:1280], r0v[:, :, 1024:1280])
        if not skip_lb2:
            nc.sync.dma_start(R0LB_sb[:], t["R0LB"][:])
            nc.sync.dma_start(R1LB_sb[:], t["R1LB"][:])

        # ---- prep: replicated pre-activations (PE) + silu/copy (ACT/DVE) ---
        # order matches downstream consumption: z0 chunks first, bias MLP
        # early enough for the first bank's bias chunk, xbc[1:] last.
        def prep_h(q):
            pp = prep_psum.tile([P, BC], F32, name=f"ph{q}", tag="pp")
            nc.tensor.matmul(pp[:], lhsT=lw1r_q[q], rhs=emb_sb,
                             start=True, stop=True)
            nc.scalar.activation(hbc[q][:], pp[:], AF.Silu, **bias_kw(q))

        def prep_x(k):
            pp = prep_psum.tile([P, BC], F32, name=f"px{k}", tag="pp")
            nc.tensor.matmul(pp[:], lhsT=W1r_sb, rhs=featv_sb[k - 1],
                             start=True, stop=True)
            nc.vector.tensor_copy(out=xbc[k][:], in_=pp[:])

        def prep_hb():
            pp = prep_psum.tile([P, BC], F32, name="phb", tag="pp")
            nc.tensor.matmul(pp[0:64, :], lhsT=bw1_sb, rhs=emb_sb,
                             start=True, stop=True)
            nc.scalar.activation(hbp_sb[0:64, :], pp[0:64, :], AF.Silu,
                                 **({} if skip_b1 else {"bias": bb1_sb}))

        # hbc0/xbc0 gate z0[0]; they run through main-pool psum slots (free
        # until the first sched tile) so the split silu does not hold the
        # prep ring, and the first 128-col slice of z0[0] -- all the first
        # main matmul needs -- flows silu-a -> copy-a -> mul-a with no
        # full-width op in between.  The rest of prep is interleaved into
        # the phase-1 q-loop below.
        pp_h0 = main_psum.tile([P, BC], F32, name="pp_h0", tag="mp")
        nc.tensor.matmul(pp_h0[:], lhsT=lw1r_q[0], rhs=emb_sb,
                         start=True, stop=True)
        nc.scalar.activation(hbc[0][:, 0:128], pp_h0[:, 0:128], AF.Silu,
                             **bias_kw(0))
        pp_x0 = main_psum.tile([P, BC], F32, name="pp_x0", tag="mp")
        nc.tensor.matmul(pp_x0[:], lhsT=W0r_sb, rhs=feats_sb,
                         start=True, stop=True)
        nc.vector.tensor_copy(out=xbc[0][:, 0:128], in_=pp_x0[:, 0:128])
        nc.vector.tensor_mul(out=z[0][0][:, 0:128], in0=hbc[0][:, 0:128],
                             in1=xbc[0][:, 0:128])
        nc.scalar.activation(hbc[0][:, 128:BC], pp_h0[:, 128:BC], AF.Silu,
                             **bias_kw(0))
        # read xbc0 straight from PSUM here: skips the SBUF copy from the
        # z0[0] critical path (the copy below only feeds z0[1..7])
        nc.vector.tensor_mul(out=z[0][0][:, 128:BC], in0=hbc[0][:, 128:BC],
                             in1=pp_x0[:, 128:BC])
        prep_h(1)
        prep_h(2)
        # z0[1]/z0[2] also read xbc0 from PSUM: they are needed within ~1us
        # (phase-1 consumes one z0 chunk per 853 ns) while the SBUF copy of
        # xbc0 below only feeds z0[3..7] and the z1..z3 builds
        for q in range(1, 3):
            nc.vector.tensor_mul(out=z[0][q][:], in0=hbc[q][:],
                                 in1=pp_x0[:])
        nc.vector.tensor_copy(out=xbc[0][:, 128:BC], in_=pp_x0[:, 128:BC])

        def ph_z(q):
            prep_h(q)
            nc.vector.tensor_mul(out=z[0][q][:], in0=hbc[q][:], in1=xbc[0][:])

        # ---- mains: bank-major, j-pair tiles, chunk-major inside a pair ----
        # (tdx, rhs_sb, src_col, ncols, bias_cols, dst_col)
        banks = {
            "00a": (0, R0_sb, 0, 512, (0, 512), 0),
            "00b": (0, R0_sb, 512, 512, (512, 1024), 512),
            "01_0": (1, R1_sb, 0, 512, None, 1024),
            "10_0": (1, R1_sb, 512, 512, None, 1536),
            "01_1": (2, R1_sb, 0, 512, None, 2048),
            "10_1": (2, R1_sb, 512, 512, None, 2560),
            "01_2": (3, R1_sb, 0, 512, None, 3072),
            "10_2": (3, R1_sb, 512, 512, None, 3584),
            "11": (0, R0_sb, 1024, 256, (1024, 1280), 4096),
            # p11 col-halves: the very last flush is only [128,128] so the
            # end-of-kernel copy+DMA+sem chain drains a minimal payload
            "11a": (0, R0_sb, 1024, 128, (1024, 1152), 4096),
            "11b": (0, R0_sb, 1152, 128, (1152, 1280), 4224),
        }
        outr = t["out"].rearrange("(jj p) n -> p jj n", p=P)
        copy_eng = [nc.scalar.copy,
                    lambda dst, src: nc.vector.tensor_copy(out=dst, in_=src)]
        ci = 0

        def mm_tile(q, js, spec, pt):
            tdx, rhs_sb, c0, ncols, bias_cols, dst = spec
            nz = 8 + (0 if skip_lb2 else 1) + (0 if bias_cols is None else 1)
            for gi, j in enumerate(js):
                nc.tensor.matmul(
                    pt[:, 512 * gi:512 * gi + ncols],
                    lhsT=z[tdx][q][:, bsl[j]],
                    rhs=rhs_sb[:, q, c0:c0 + ncols],
                    start=(q == 0), stop=(q == 7 and nz == 8))

        def fin_tile(name, js, spec, pt):
            tdx, rhs_sb, c0, ncols, bias_cols, dst = spec
            nonlocal ci
            if not skip_lb2:
                rlb = R0LB_sb if rhs_sb is R0_sb else R1LB_sb
                for gi, j in enumerate(js):
                    nc.tensor.matmul(
                        pt[:, 512 * gi:512 * gi + ncols],
                        lhsT=xbc[tdx][0:16, bsl[j]],
                        rhs=rlb[:, c0:c0 + ncols],
                        start=False, stop=(bias_cols is None))
            if bias_cols is not None:
                for gi, j in enumerate(js):
                    nc.tensor.matmul(
                        pt[:, 512 * gi:512 * gi + ncols],
                        lhsT=hbp_sb[:, bsl[j]],
                        rhs=BB_sb[:, bias_cols[0]:bias_cols[1]],
                        start=False, stop=True)
            ng = len(js)
            st = spool.tile([P, 512 * ng], F32, name=f"st{name}", tag="st")
            if ncols == 512:
                copy_eng[ci % 2](st[:], pt[:, 0:512 * ng])
            elif ng == 1:
                copy_eng[ci % 2](st[:, 0:ncols], pt[:, 0:ncols])
            else:
                copy_eng[0](st[:, 0:ncols], pt[:, 0:ncols])
                copy_eng[1](st[:, 512:512 + ncols], pt[:, 512:512 + ncols])
            ci += 1
            if ng == 2:
                src = st[:].rearrange("p (g n) -> p g n", g=2)[:, :, 0:ncols]
                nc.sync.dma_start(outr[:, js[0]:js[0] + 2, dst:dst + ncols],
                                  src)
            else:
                nc.sync.dma_start(outr[:, js[0], dst:dst + ncols],
                                  st[:, 0:ncols])

        # phase 1: p00a j-pairs (0,1) and (2,3) interleaved q-major -- two
        # tiles consume z0[q] at the silu production rate (no z stall) --
        # with the remaining prep matmuls slotted between q-steps
        ptA = main_psum.tile([P, 1024], F32, name="ptA", tag="mp")
        ptB = main_psum.tile([P, 1024], F32, name="ptB", tag="mp")
        prep_rest = [lambda q=q: ph_z(q) for q in range(3, 8)]
        prep_rest += [prep_hb, lambda: prep_x(1), lambda: prep_x(2),
                      lambda: prep_x(3)]
        for q in range(8):
            mm_tile(q, (0, 1), banks["00a"], ptA)
            mm_tile(q, (2, 3), banks["00a"], ptB)
            take = 1 if q < 7 else len(prep_rest)
            for _ in range(take):
                if prep_rest:
                    prep_rest.pop(0)()
        fin_tile("00a_01", (0, 1), banks["00a"], ptA)
        fin_tile("00a_23", (2, 3), banks["00a"], ptB)
        for tdx in range(1, 4):
            for q in range(8):
                nc.vector.tensor_mul(out=z[tdx][q][:], in0=hbc[q][:],
                                     in1=xbc[tdx][:])

        # remaining banks as j-pair tiles; p11 last, its final two j's as
        # single-group tiles so the kernel tail flushes only 128 KB
        sched = [("00b", (0, 1)), ("00b", (2, 3)),
                 ("01_0", (0, 1)), ("01_0", (2, 3)),
                 ("10_0", (0, 1)), ("10_0", (2, 3)),
                 ("01_1", (0, 1)), ("01_1", (2, 3)),
                 ("10_1", (0, 1)), ("10_1", (2, 3)),
                 ("01_2", (0, 1)), ("01_2", (2, 3)),
                 ("10_2", (0, 1)), ("10_2", (2, 3)),
                 ("11", (0, 1)), ("11", (2,)), ("11", (3,))]
        for name, js in sched:
            pt = main_psum.tile([P, 512 * len(js)], F32,
                                name=f"pt{name}_{js[0]}", tag="mp")
            for q in range(8):
                mm_tile(q, js, banks[name], pt)
            fin_tile(f"{name}_{js[0]}", js, banks[name], pt)


def _prepare(inputs):
    f32 = np.float32
    feat = np.asarray(inputs["feat"], dtype=f32)
    node_emb = np.asarray(inputs["node_emb"], dtype=f32)
    W0 = np.asarray(inputs["W0"], f32)
    W1 = np.asarray(inputs["W1"], f32)
    lw1 = np.asarray(inputs["lw1"], f32)
    lb1 = np.asarray(inputs["lb1"], f32)
    lw2 = np.asarray(inputs["lw2"], f32)
    lb2 = np.asarray(inputs["lb2"], f32)
    bw1 = np.asarray(inputs["bw1"], f32)
    bb1 = np.asarray(inputs["bb1"], f32)
    bw2 = np.asarray(inputs["bw2"], f32)
    bb2 = np.asarray(inputs["bb2"], f32)

    s16 = np.float32(1.0 / 16.0)
    sC = np.float32(C3 / 16.0)

    lw2p = np.concatenate([lw2, lb2[None]], axis=0)           # [65, 36864]
    M00 = lw2p[:, :16384].reshape(1040, 1024) * s16
    M11 = lw2p[:, 16384:20480].reshape(1040, 256) * sC
    M01 = lw2p[:, 20480:28672].reshape(1040, 512) * sC
    M10 = lw2p[:, 28672:36864].reshape(1040, 512) * sC
    R0f = np.concatenate([M00, M11], axis=1)                  # [1040, 1280]
    R1f = np.concatenate([M01, M10], axis=1)                  # [1040, 1024]
    R0 = np.ascontiguousarray(R0f[:1024]).astype(BF16)
    R1 = np.ascontiguousarray(R1f[:1024]).astype(BF16)
    R0LB = np.ascontiguousarray(R0f[1024:]).astype(BF16)
    R1LB = np.ascontiguousarray(R1f[1024:]).astype(BF16)
    BBf = np.concatenate([bw2, bb2[None]], axis=0)            # [65, 1280]
    BB = np.ascontiguousarray(
        np.concatenate([BBf[:, :1024] * s16, BBf[:, 1024:] * sC], axis=1)
    ).astype(BF16)

    W0s = W0 * np.float32(1.0 / np.sqrt(128.0))               # [128, 16]
    W1s = W1 * np.float32(1.0 / 8.0)                          # [64, 16]
    W0r = np.ascontiguousarray(np.tile(W0s, (1, 8))).astype(BF16)
    W1r = np.ascontiguousarray(np.tile(W1s, (1, 8))).astype(BF16)
    # lw1_rep[k, 128q + 16c8 + w] = lw1[k, 8q + c8]
    lw1r = np.repeat(lw1.reshape(P, 8, 8, 1), 16, axis=3).reshape(P, 1024)
    lw1r = np.ascontiguousarray(lw1r).astype(BF16)
    # lb1_rep[16c8 + w, q] = lb1[8q + c8]
    lb1r = np.repeat(lb1.reshape(8, 8).T[:, None, :], 16, axis=1).reshape(P, 8)
    lb1r = np.ascontiguousarray(lb1r.astype(f32))
    bw1q = np.ascontiguousarray(bw1).astype(BF16)

    skip_lb2 = not bool(np.any(lb2))
    skip_b1 = not (bool(np.any(lb1)) or bool(np.any(bb1)))

    WF = np.zeros((P, 9), f32)
    WF[:, 0:8] = lb1r
    WF[0:64, 8] = bb1

    in_maps = []
    for i in range(N_CORES):
        sl = slice(i * BC, (i + 1) * BC)
        fs = feat[sl]
        featT = np.concatenate(
            [fs[:, :128], fs[:, 128::3], fs[:, 129::3], fs[:, 130::3]],
            axis=1).T.astype(BF16)                             # [320, BC]
        embT = node_emb[sl].T.astype(BF16)
        WS0a = np.ascontiguousarray(
            np.concatenate([lw1r[:, 0:128], embT], axis=1))    # [128, 640]
        WS0b = np.ascontiguousarray(
            np.concatenate([W0r, featT[0:128]], axis=1))       # [128, 640]
        WS1a = np.ascontiguousarray(lw1r[:, 128:256])          # [128, 128]
        WS1b = np.ascontiguousarray(
            np.concatenate([lw1r[:, 256:1024], bw1q], axis=1))  # [128, 832]
        WS2 = np.ascontiguousarray(
            np.concatenate([W1r, featT[128:192], featT[192:256],
                            featT[256:320]], axis=1))          # [64, 1664]
        m = {
            "WS0a": WS0a, "WS0b": WS0b, "WS1a": WS1a, "WS1b": WS1b,
            "WS2": WS2, "WF": WF,
            "R0": R0, "R1": R1, "BB": BB,
        }
        if not skip_lb2:
            m["R0LB"] = R0LB
            m["R1LB"] = R1LB
        in_maps.append(m)
    return in_maps, skip_lb2, skip_b1


def _expand(c):
    """[B, 4352] compact -> [B, 80, 80] full block structure."""
    B = c.shape[0]
    out = np.zeros((B, 80, 80), np.float32)
    out[:, :32, :32] = c[:, 0:1024].reshape(B, 32, 32)
    mid = c[:, 1024:4096].reshape(B, 3, 2, 512)
    out[:, :32, 32:] = (mid[:, :, 0, :].reshape(B, 3, 32, 16)
                        .transpose(0, 2, 3, 1).reshape(B, 32, 48))
    out[:, 32:, :32] = (mid[:, :, 1, :].reshape(B, 3, 16, 32)
                        .transpose(0, 2, 1, 3).reshape(B, 48, 32))
    r11 = c[:, 4096:4352].reshape(B, 16, 16)
    v = out[:, 32:, 32:].reshape(B, 16, 3, 16, 3)
    for i in range(3):
        v[:, :, i, :, i] = r11
    return out


def run(inputs, mode=None, trace=False):
    """Build (cached), run on 8 cores, gather. Returns (out, results)."""
    in_maps, skip_lb2, skip_b1 = _prepare(inputs)
    key = (skip_lb2, skip_b1)
    if key not in _CACHE:
        _CACHE[key] = _build_program(skip_lb2, skip_b1)
    nc = _CACHE[key]

    from concourse.bass_utils import run_bass_kernel_spmd
    res = run_bass_kernel_spmd(nc, in_maps, list(range(N_CORES)), trace=trace)
    c = np.concatenate(
        [np.asarray(res.results[i]["out"]) for i in range(N_CORES)], axis=0)
    return _expand(c.astype(np.float32)), res


def kernel(**inputs):
    out, _ = run(inputs)
    return out
